# revision 29
# baseline (speedup 1.0000x reference)
"""Two-layer GCN (message passing) on 8 Trainium2 NeuronCores.

Strategy:
  - Shard dst nodes across 8 cores (12500 each, 98 blocks of 128 slots).
  - Edges partitioned by dst owner; per core, each (dst-block, src-group) pair
    gets a fixed 640-slot run (5 chunks of 128 edges, padded with null edges).
    src-groups are 25000-node ranges so gather indices fit in int16.
  - Layer 1: sharded GEMM x@W1 -> AllGather bf16 h table [100000,128] ->
    per-superblock dma_gather of source rows -> fused DVE tensor_scalar builds
    a weighted one-hot [128 edges, 128 nodes] per chunk -> PE matmul
    accumulates aggT = msg^T @ onehot in PSUM -> relu -> @W2 -> h2 shard ->
    AllGather h2 table -> Layer 2 repeats gather/one-hot/matmul -> out.
  - Output written feature-major per core; host transposes/concatenates.
"""

import os
import sys

import numpy as np

for _p in ("/opt/trn_rl_repo", "/root/.axon_site/_ro/trn_rl_repo"):
    if os.path.isdir(_p) and _p not in sys.path:
        sys.path.append(_p)

import ml_dtypes  # noqa: E402

import concourse.bacc as bacc  # noqa: E402
import concourse.mybir as mybir  # noqa: E402
from concourse import library_config, tile  # noqa: E402
from concourse.bass_utils import run_bass_kernel_spmd  # noqa: E402

BF16 = ml_dtypes.bfloat16

# ---- problem constants (nn_BaselineGCN: N=100000, E=1600000, 128->128->64) ----
N_NODES = 100000
N_EDGES = 1600000
F_IN = 128
F_HID = 128
F_OUT = 64

NCORES = 8
NPC = N_NODES // NCORES          # 12500 dst nodes per core
BLK = 128                        # nodes per dst block
NBLK = (NPC + BLK - 1) // BLK    # 98 blocks per core
SLOTPC = NBLK * BLK              # 12544 node slots per core (44 dummies)
NG = 4                           # src groups (int16 index limit)
G1 = N_NODES // NG               # 25000 rows per L1 gather group
CPB_G = 5                        # chunks per (block, group) run
RUNSLOTS = CPB_G * BLK           # 640 edge slots per run
KBLK = CPB_G * NG                # 20 chunks per block
SBB = 7                          # blocks per superblock
NSB = NBLK // SBB                # 14 superblocks
CH_SB = SBB * KBLK               # 140 chunks per superblock
SLOT_SB = CH_SB * 128            # 17920 edge slots per superblock
NCHUNK = NBLK * KBLK             # 1960 chunks per core per layer
NSLOT = NCHUNK * 128             # 250880 edge slots per core per layer
H2ROWS = NCORES * SLOTPC         # 100352 rows in h2 table
G2 = H2ROWS // NG                # 25088 rows per L2 gather group
IDXCOLS = NSLOT // 16            # idx16 tensor free dim
IDXCOLS_SB = SLOT_SB // 16       # 1120 per superblock
IDXCOLS_G = RUNSLOTS * SBB // 16  # 280 idx cols per (superblock, group) call
NIDX_CALL = RUNSLOTS * SBB       # 4480 indices per gather call

_CACHE: dict = {}


def _wrap_idx16(v: np.ndarray) -> np.ndarray:
    """Pack indices for dma_gather: index i -> [i%16, i//16], replicated
    across the 8 groups of 16 partitions."""
    block = v.astype(np.int16).reshape(-1, 16).T  # [16, n/16]
    return np.tile(block, (8, 1))                 # [128, n/16]


def _prep_core(c: int, src: np.ndarray, dst: np.ndarray, ew: np.ndarray):
    """Edge-slot layout for core c. Returns idx16_l1, idx16_l2, dstloc, wcol."""
    m = (dst // NPC) == c
    es = src[m].astype(np.int64)
    ed = (dst[m] - c * NPC).astype(np.int64)
    w = ew[m].astype(np.float32)

    g = es // G1
    b = ed // BLK
    loc = ed % BLK
    run = b * NG + g
    counts = np.bincount(run, minlength=NBLK * NG)
    if counts.max() > RUNSLOTS:
        raise RuntimeError(
            f"core {c}: run overflow {counts.max()} > {RUNSLOTS}; "
            f"increase CPB_G"
        )

    order = np.argsort(run, kind="stable")
    run_s = run[order]
    start_of_run = np.searchsorted(run_s, np.arange(NBLK * NG))
    pos = np.arange(len(es)) - start_of_run[run_s]
    bs, gs = b[order], g[order]
    run_base = (bs // SBB) * SLOT_SB + gs * (SBB * RUNSLOTS) + (bs % SBB) * RUNSLOTS
    slot = run_base + pos

    idx1 = np.zeros(NSLOT, np.int64)
    idx2 = np.zeros(NSLOT, np.int64)
    hiw = np.zeros((NSLOT, 16), np.float32)
    lo8 = np.zeros((NSLOT, 8), np.float32)

    es_s = es[order]
    idx1[slot] = es_s - gs * G1
    o = es_s // NPC
    idx2[slot] = (o % 2) * SLOTPC + (es_s - o * NPC)
    loc_s = loc[order]
    hiw[slot, loc_s // 8] = w[order]
    lo8[slot, loc_s % 8] = 1.0

    def chunk_major(a, k):
        return np.ascontiguousarray(
            a.reshape(NCHUNK, 128, k).transpose(1, 0, 2).reshape(
                128, NCHUNK * k)).astype(BF16)

    return (
        _wrap_idx16(idx1),
        _wrap_idx16(idx2),
        chunk_major(hiw, 16),
        chunk_major(lo8, 8),
    )


def _build_program():
    dbg_nsb = int(os.environ.get("KERNEL_DBG_NSB", str(NSB)))
    dbg_nogather = bool(int(os.environ.get("KERNEL_DBG_NOGATHER", "0")))
    dbg_nocoll = bool(int(os.environ.get("KERNEL_DBG_NOCOLL", "0")))
    nc = bacc.Bacc("TRN2", target_bir_lowering=False, debug=False,
                   num_devices=NCORES, num_swdge_queues=4,
                   dynamic_dma_scratch_size=24576)

    xT_d = nc.dram_tensor("xT", [F_IN, NPC], mybir.dt.bfloat16,
                          kind="ExternalInput")
    W1_d = nc.dram_tensor("W1b", [F_IN, F_HID], mybir.dt.bfloat16,
                          kind="ExternalInput")
    W2_d = nc.dram_tensor("W2b", [F_HID, F_OUT], mybir.dt.bfloat16,
                          kind="ExternalInput")
    idx1_d = nc.dram_tensor("idx1", [128, IDXCOLS], mybir.dt.int16,
                            kind="ExternalInput")
    idx2_d = nc.dram_tensor("idx2", [128, IDXCOLS], mybir.dt.int16,
                            kind="ExternalInput")
    hiw_d = nc.dram_tensor("hiw", [128, NCHUNK * 16], mybir.dt.bfloat16,
                           kind="ExternalInput")
    lo8_d = nc.dram_tensor("lo8", [128, NCHUNK * 8], mybir.dt.bfloat16,
                           kind="ExternalInput")
    out_d = nc.dram_tensor("outT", [F_OUT, SLOTPC], mybir.dt.float32,
                           kind="ExternalOutput")

    with tile.TileContext(nc) as tc:
        nc.gpsimd.load_library(library_config.mlp)
        with (
            tc.tile_pool(name="dram", bufs=1, space="DRAM") as dram,
            tc.tile_pool(name="const", bufs=1) as constp,
            tc.tile_pool(name="idxp", bufs=2) as idxp,
            tc.tile_pool(name="msgp", bufs=2) as msgp,
            tc.tile_pool(name="ohp", bufs=2) as ohp,
            tc.tile_pool(name="smallp", bufs=4) as smallp,
            tc.tile_pool(name="psagg", bufs=2, space="PSUM") as psagg,
            tc.tile_pool(name="psgemm", bufs=2, space="PSUM") as psgemm,
        ):
            h_loc = dram.tile([NPC, F_HID], mybir.dt.bfloat16)
            h_full = dram.tile([N_NODES, F_HID], mybir.dt.bfloat16,
                               addr_space="Shared")
            h2_loc = dram.tile([SLOTPC, 128], mybir.dt.bfloat16)
            h2_full = dram.tile([H2ROWS, 128], mybir.dt.bfloat16,
                                addr_space="Shared")

            w1_t = constp.tile([F_IN, F_HID], mybir.dt.bfloat16)
            nc.sync.dma_start(w1_t[:], W1_d[:])
            w2_t = constp.tile([F_HID, F_OUT], mybir.dt.bfloat16)
            nc.sync.dma_start(w2_t[:], W2_d[:])

            # ---- GEMM1: h_loc = (xT)^T @ W1, 128-node tiles ----
            with tc.tile_pool(name="xtp", bufs=2) as xtp:
                XTW = 2048  # cols per load tile (16 blocks exactly)
                for t0 in range(0, NBLK, XTW // BLK):
                    ncols = min(XTW, NPC - t0 * BLK)
                    xt_t = xtp.tile([F_IN, XTW], mybir.dt.bfloat16)
                    nc.sync.dma_start(
                        xt_t[:, :ncols],
                        xT_d[:, t0 * BLK:t0 * BLK + ncols])
                    hw_t = xtp.tile([128, XTW], mybir.dt.bfloat16, tag="hw")
                    for tt in range(0, ncols, BLK):
                        nr = min(BLK, ncols - tt)
                        ps = psgemm.tile([128, F_HID], mybir.dt.float32,
                                         tag="gemm")
                        nc.tensor.matmul(
                            ps[:nr, :], xt_t[:, tt:tt + nr], w1_t[:],
                            start=True, stop=True,
                        )
                        nc.scalar.activation(
                            hw_t[:nr, tt:tt + F_HID], ps[:nr, :],
                            mybir.ActivationFunctionType.Copy)
                    ncf = (ncols // BLK) * BLK
                    if ncf:
                        nc.scalar.dma_start(
                            h_loc[t0 * BLK:t0 * BLK + ncf, :]
                            .rearrange("(t p) f -> p t f", p=BLK),
                            hw_t[:, :ncf].rearrange("p (t f) -> p t f",
                                                    f=F_HID))
                    if ncols > ncf:
                        nr = ncols - ncf
                        nc.scalar.dma_start(
                            h_loc[t0 * BLK + ncf:t0 * BLK + ncols, :],
                            hw_t[:nr, ncf:ncf + F_HID])

            if dbg_nocoll:
                nc.sync.dma_start(h_full[:NPC, :], h_loc[:])
            else:
                nc.gpsimd.collective_compute(
                    "AllGather",
                    mybir.AluOpType.bypass,
                    ins=[h_loc.opt()],
                    outs=[h_full.opt()],
                    replica_groups=[list(range(NCORES))],
                )

            # ---- layer loops ----
            for layer in (1, 2):
                idx_d = idx1_d if layer == 1 else idx2_d
                gsz = G1 if layer == 1 else G2
                table = h_full if layer == 1 else h2_full
                fmm = F_HID if layer == 1 else F_OUT

                for sb in range(dbg_nsb):
                    idx_t = idxp.tile([128, IDXCOLS_SB], mybir.dt.int16)
                    nc.sync.dma_start(
                        idx_t[:],
                        idx_d[:, sb * IDXCOLS_SB:(sb + 1) * IDXCOLS_SB])
                    msg_t = msgp.tile([128, CH_SB, 128], mybir.dt.bfloat16)
                    dbg_ngather = int(os.environ.get("KERNEL_DBG_NGATHER",
                                                     str(NG)))
                    if not dbg_nogather:
                        for g in range(dbg_ngather):
                            nc.gpsimd.dma_gather(
                                msg_t[:, g * (CH_SB // NG):(g + 1) * (CH_SB // NG), :],
                                table[g * gsz:(g + 1) * gsz, :],
                                idx_t[:, g * IDXCOLS_G:(g + 1) * IDXCOLS_G],
                                NIDX_CALL, NIDX_CALL, 128,
                                single_packet=False, queue_num=g,
                            )
                    else:
                        nc.vector.memset(msg_t[:, 0, :], 0.0)

                    # whole-superblock weighted one-hot build: one DVE op,
                    # oh[p, c, hi*8+lo] = hiw[p, c, hi] * lo8[p, c, lo]
                    hiw_t = idxp.tile([128, CH_SB * 16], mybir.dt.bfloat16,
                                      tag="hiw")
                    nc.sync.dma_start(
                        hiw_t[:],
                        hiw_d[:, sb * CH_SB * 16:(sb + 1) * CH_SB * 16])
                    lo_t = idxp.tile([128, CH_SB * 8], mybir.dt.bfloat16,
                                     tag="lo8")
                    nc.sync.dma_start(
                        lo_t[:],
                        lo8_d[:, sb * CH_SB * 8:(sb + 1) * CH_SB * 8])
                    oh_t = ohp.tile([128, CH_SB, 128], mybir.dt.bfloat16)
                    HC = CH_SB // 2
                    for half in range(2):
                        nc.vector.tensor_tensor(
                            oh_t[:, half * HC:(half + 1) * HC, :]
                            .rearrange("p c (h l) -> p c h l", l=8),
                            (hiw_t[:, half * HC * 16:(half + 1) * HC * 16]
                             .rearrange("p (c h) -> p c h", h=16)
                             .unsqueeze(3).broadcast_to((128, HC, 16, 8))),
                            (lo_t[:, half * HC * 8:(half + 1) * HC * 8]
                             .rearrange("p (c l) -> p c l", l=8)
                             .unsqueeze(2).broadcast_to((128, HC, 16, 8))),
                            mybir.AluOpType.mult)

                    psA = psagg.tile([128, 512], mybir.dt.float32, tag="psA")
                    psB = psagg.tile([128, 512], mybir.dt.float32, tag="psB")

                    def agg_slice(bi, psA=psA, psB=psB, fmm=fmm):
                        pst = psA if bi < 4 else psB
                        j = bi if bi < 4 else bi - 4
                        return pst[:fmm, j * 128:(j + 1) * 128]

                    # g-major (chunks in gather order, so matmuls of group g
                    # start as soon as gather g lands). PSUM has_written clear
                    # on start=True is bank-wide, so exactly one start per
                    # bank per superblock; per-element has_written then makes
                    # each block-slice's first write an overwrite.
                    for g in range(NG):
                        for bi in range(SBB):
                            for k in range(CPB_G):
                                ch = g * (CH_SB // NG) + bi * CPB_G + k
                                nc.tensor.matmul(
                                    agg_slice(bi),
                                    msg_t[:, ch, :fmm], oh_t[:, ch, :],
                                    start=(g == 0 and k == 0 and bi in (0, 4)),
                                    stop=(g == NG - 1 and k == CPB_G - 1
                                          and bi in (3, 6)),
                                    skip_group_check=True,
                                )

                    if layer == 1:
                        h2w_t = smallp.tile([128, SBB * 128],
                                            mybir.dt.bfloat16, tag="h2w")
                        nc.vector.memset(h2w_t[:], 0.0)
                        for bi in range(SBB):
                            relu_t = smallp.tile([128, 128],
                                                 mybir.dt.bfloat16, tag="relu")
                            nc.scalar.activation(
                                relu_t[:], agg_slice(bi),
                                mybir.ActivationFunctionType.Relu)
                            h2ps = psgemm.tile([128, F_OUT], mybir.dt.float32,
                                               tag="gemm")
                            nc.tensor.matmul(h2ps[:], relu_t[:], w2_t[:],
                                             start=True, stop=True)
                            nc.scalar.activation(
                                h2w_t[:, bi * 128:bi * 128 + F_OUT], h2ps[:],
                                mybir.ActivationFunctionType.Copy)
                        b0 = sb * SBB
                        nc.scalar.dma_start(
                            h2_loc[b0 * BLK:(b0 + SBB) * BLK, :]
                            .rearrange("(b p) f -> p b f", p=BLK),
                            h2w_t[:].rearrange("p (b f) -> p b f", f=128))
                    else:
                        ow_t = smallp.tile([F_OUT, SBB * 128],
                                           mybir.dt.float32, tag="ow")
                        for bi in range(SBB):
                            nc.scalar.activation(
                                ow_t[:, bi * 128:(bi + 1) * 128],
                                agg_slice(bi),
                                mybir.ActivationFunctionType.Copy)
                        nc.scalar.dma_start(
                            out_d[:, sb * SBB * BLK:(sb + 1) * SBB * BLK],
                            ow_t[:])

                if layer == 1:
                    if dbg_nocoll:
                        nc.sync.dma_start(h2_full[:SLOTPC, :], h2_loc[:])
                    else:
                        nc.gpsimd.collective_compute(
                            "AllGather",
                            mybir.AluOpType.bypass,
                            ins=[h2_loc.opt()],
                            outs=[h2_full.opt()],
                            replica_groups=[list(range(NCORES))],
                        )

    nc.compile()
    return nc


def kernel(x, W1, W2, edge_weight, edge_index):
    x = np.asarray(x)
    W1 = np.asarray(W1)
    W2 = np.asarray(W2)
    ew = np.asarray(edge_weight)
    ei = np.asarray(edge_index)
    src, dst = ei[0].astype(np.int64), ei[1].astype(np.int64)

    if "nc" not in _CACHE:
        _CACHE["nc"] = _build_program()
    nc = _CACHE["nc"]

    w1b = W1.astype(BF16)
    w2b = W2.astype(BF16)

    in_maps = []
    for c in range(NCORES):
        idx1, idx2, hiw, lo8 = _prep_core(c, src, dst, ew)
        in_maps.append({
            "xT": np.ascontiguousarray(
                x[c * NPC:(c + 1) * NPC].T).astype(BF16),
            "W1b": w1b,
            "W2b": w2b,
            "idx1": idx1,
            "idx2": idx2,
            "hiw": hiw,
            "lo8": lo8,
        })

    trace = bool(int(os.environ.get("KERNEL_TRACE", "0")))
    res = run_bass_kernel_spmd(nc, in_maps, core_ids=list(range(NCORES)),
                               trace=trace)
    _CACHE["last_result"] = res

    out = np.empty((N_NODES, F_OUT), np.float32)
    for c in range(NCORES):
        out[c * NPC:(c + 1) * NPC] = res.results[c]["outT"].T[:NPC]
    return out

